# revision 29
# baseline (speedup 1.0000x reference)
"""Trainium2 Bass kernel for nn_KG_EdgeAtt_new (sparse windowed attention).

Sharding: pure data-parallel over batch B=32 across 8 NeuronCores (4
conversations per core). Weights replicated.

The end-to-end metric is dominated by host->device transfer, so the design
minimizes shipped bytes (133MB -> ~48MB round trip vs the bf16 baseline):
  - knowledge / node_features are host-normalized (cosine similarity is
    scale-invariant, so per-vector norms fold away -> no separate norm
    tensors) and shipped as fp8_e4m3 with power-of-2 scales: unit vectors
    x16, weights x256 (entries ~N(0, 1/D) are subnormal in e4m3 unscaled;
    the scales cancel exactly inside the device-side normalizations).
  - output is returned as a bf16 band [L, BPC, 28]: only |j-k|<=10 can be
    nonzero, and 8 consecutive rows share one 28-wide window so 14 DMAs
    cover it; the host scatters it back to the full [BPC, L, L].
  - a persistent jit runner avoids per-call retracing, and the donated
    output buffers are the previous call's device-resident outputs, so no
    zero-buffers ship per call.
End-to-end rel err vs the f32 reference: 1.05e-2 (gate 2e-2).

Math (per batch b):
  semantic:   S = W_sem @ nf; cos(nf_j, S_k); score = 1 - acos(clip(cos))/pi;
              windowed softmax -> alphas_sem
  contextual: A_n = W_con @ K_n; cos(K_nj, A_nk) (the anew affinity scale is
              strictly positive so it cancels in cosine similarity);
              alphas_con = 10 * sum_n |cos| (windowed)
  out = 0.5*alphas_sem + 0.5*alphas_con, masked.
"""

import sys

sys.path.insert(0, "/opt/trn_rl_repo")

import math
from contextlib import ExitStack
from types import SimpleNamespace

import ml_dtypes
import numpy as np

import concourse.bass as bass
import concourse.bacc as bacc
import concourse.mybir as mybir
import concourse.tile as tile
from concourse.bass import ds, ts

BF = mybir.dt.bfloat16
F32 = mybir.dt.float32
FP8 = mybir.dt.float8e4
AF = mybir.ActivationFunctionType
OP = mybir.AluOpType
AX = mybir.AxisListType

B, L, G, N, D = 32, 110, 512, 40, 300
NCORES = 8
BPC = B // NCORES  # 4
WP, WF = 10, 10
GB = 8                      # output rows per band DMA
BW = WP + WF + GB           # 28: window width covering GB consecutive rows' bands
CLIP = 1.0 - 1e-6
NG = 4                      # knowledge slots per matmul group (free dim 440)
NG2 = 2 * NG                # slots per DMA (two groups per transfer)
NGRP = N // NG              # 10
BL = BPC * L                # 440
DT = [128, 128, 44]         # 300 split into partition tiles
P = 128
NEG = 1.0e4                 # masked-logit offset (exp(-1e4) == 0 in f32)

# host-side fp8 scales (power-of-2; cancel exactly on device)
KS = 16.0                   # knowledge / node_features (unit vectors ~0.06)
WS = 256.0                  # weights (~N(0, 1/D) are subnormal in e4m3)

# acos(x) ~= sqrt(1-x) * (a0 + a1 x + a2 x^2 + a3 x^3), x in [0,1] (A&S 4.4.45)
# coefficients pre-divided by pi: score = 0.5 + sg*(0.5 - sqrt(1-t)*poly(t))
A0, A1, A2, A3 = (c / math.pi for c in (1.5707288, -0.2121144, 0.0742610, -0.0187293))


def _build_nc():
    nc = bacc.Bacc("TRN2", target_bir_lowering=False, debug=False, num_devices=NCORES)
    kT = nc.declare_dram_parameter("kT", [BPC, D, N, L], FP8, isOutput=False)
    nfT = nc.declare_dram_parameter("nfT", [G, BPC, L], FP8, isOutput=False)
    wsemT = nc.declare_dram_parameter("wsemT", [G, G], FP8, isOutput=False)
    wcon = nc.declare_dram_parameter("wcon", [D, D], FP8, isOutput=False)
    fmask = nc.declare_dram_parameter("fmask", [BPC, L, L], FP8, isOutput=False)
    out = nc.declare_dram_parameter("out", [L, BPC, BW], BF, isOutput=True)

    with tile.TileContext(nc) as tc, ExitStack() as ctx:
        _emit(ctx, tc, nc, kT, nfT, wsemT, wcon, fmask, out)
    nc.compile()
    return nc


def _emit(ctx, tc, nc, kT, nfT, wsemT, wcon, fmask, out):
    consts = ctx.enter_context(tc.tile_pool(name="consts", bufs=1))

    ones_bf = consts.tile([P, P], BF, tag="ones")
    nc.gpsimd.memset(ones_bf[:], 1.0)

    wsem_sb = []
    for i in range(4):
        t = consts.tile([P, G], FP8, tag=f"wsem{i}")
        nc.sync.dma_start(out=t[:], in_=wsemT[ts(i, P), :])
        wsem_sb.append(t)
    wcon_sb = []
    for i, d_ in enumerate(DT):
        t = consts.tile([P, D], FP8, tag=f"wcon{i}")
        nc.sync.dma_start(out=t[:d_], in_=wcon[ds(i * 128, d_), :])
        wcon_sb.append(t)
    nfT_sb = []
    for i in range(4):
        t = consts.tile([P, BL], FP8, tag=f"nfT{i}")
        nc.sync.dma_start(out=t[:], in_=nfT[ts(i, P)].rearrange("g b l -> g (b l)"))
        nfT_sb.append(t)
    fm_sb, fmb_sb = [], []
    for b in range(BPC):
        t8 = consts.tile([L, L], FP8, tag=f"fm8{b}")
        nc.sync.dma_start(out=t8[:], in_=fmask[b])
        t = consts.tile([L, L], F32, tag=f"fm{b}")
        nc.scalar.copy(out=t[:], in_=t8[:])
        fm_sb.append(t)
        # fmb = 0.5*fm - NEG*(1-fm): masked logit floor plus the +0.5 of the score
        u = consts.tile([L, L], F32, tag=f"fmb{b}")
        nc.vector.tensor_scalar(out=u[:], in0=t[:], scalar1=0.5 + NEG, scalar2=-NEG,
                                op0=OP.mult, op1=OP.add)
        fmb_sb.append(u)

    # ---------------- semantic head: S, norms, cos ----------------
    sem = ctx.enter_context(tc.tile_pool(name="sem", bufs=1))
    cos_sb = []
    with tc.tile_pool(name="psS", bufs=4, space="PSUM") as psS, \
         tc.tile_pool(name="psNs", bufs=1, space="PSUM") as psNs, \
         tc.tile_pool(name="psM", bufs=2, space="PSUM") as psM:
        s_ps, scp = [], []
        for gt in range(4):
            pt = psS.tile([P, BL], F32, tag="sps")
            for tt_ in range(4):
                nc.tensor.matmul(pt[:], lhsT=wsem_sb[tt_][:, ts(gt, P)],
                                 rhs=nfT_sb[tt_][:], start=(tt_ == 0), stop=(tt_ == 3))
            s_ps.append(pt)
            c = consts.tile([P, BL], FP8, tag=f"scp{gt}")
            nc.scalar.copy(out=c[:], in_=pt[:])
            scp.append(c)
        ssq = []
        for gt in range(4):
            q = sem.tile([P, BL], BF, tag=f"ssq{gt}")
            nc.vector.tensor_mul(q[:], scp[gt][:], scp[gt][:])
            ssq.append(q)
        pn = psNs.tile([P, BL], F32, tag="pns")
        for gt in range(4):
            nc.tensor.matmul(pn[:], lhsT=ones_bf[:], rhs=ssq[gt][:],
                             start=(gt == 0), stop=(gt == 3))
        rna_f = sem.tile([P, BL], F32, tag="rnaf")
        nc.vector.reciprocal(rna_f[:], pn[:])
        # rna = 1/(KS*||S||): folds the node-feature fp8 scale away
        rna = consts.tile([P, BL], F32, tag="rna")
        nc.scalar.activation(rna[:], rna_f[:], AF.Sqrt, scale=1.0 / (KS * KS))

        for b in range(BPC):
            pm = psM.tile([L, L], F32, tag="pm")
            for gt in range(4):
                nc.tensor.matmul(pm[:], lhsT=nfT_sb[gt][:, ts(b, L)],
                                 rhs=scp[gt][:, ts(b, L)], start=(gt == 0), stop=(gt == 3))
            cz = consts.tile([L, L], F32, tag=f"cos{b}")
            nc.vector.tensor_mul(cz[:], pm[:], rna[:L, ts(b, L)])
            cos_sb.append(cz)

    # ---------------- contextual branch + per-batch tail ----------------
    tc.strict_bb_all_engine_barrier()
    kp = ctx.enter_context(tc.tile_pool(name="kp", bufs=4))
    ap = ctx.enter_context(tc.tile_pool(name="ap", bufs=6))
    sq = ctx.enter_context(tc.tile_pool(name="sq", bufs=6))
    rp = ctx.enter_context(tc.tile_pool(name="rp", bufs=2))
    cp = ctx.enter_context(tc.tile_pool(name="cp", bufs=3))
    accp = ctx.enter_context(tc.tile_pool(name="accp", bufs=1))
    semp = ctx.enter_context(tc.tile_pool(name="semp", bufs=2))
    obp = ctx.enter_context(tc.tile_pool(name="obp", bufs=1))
    oball = obp.tile([L, BPC * L], BF, tag="oball", name="oball")
    psA = ctx.enter_context(tc.tile_pool(name="psA", bufs=3, space="PSUM"))
    psN = ctx.enter_context(tc.tile_pool(name="psN", bufs=2, space="PSUM"))
    psC = ctx.enter_context(tc.tile_pool(name="psC", bufs=3, space="PSUM"))

    for b in range(BPC):
        acc = accp.tile([L, NG * L], F32, tag=f"acc{b}")
        nc.gpsimd.memset(acc[:], 0.0)
        for gp in range(NGRP // 2):
            ktp = []
            for i, d_ in enumerate(DT):
                t = kp.tile([P, NG2 * L], FP8, tag="kt")
                nc.sync.dma_start(
                    out=t[:d_],
                    in_=kT[b, ds(i * 128, d_), ds(gp * NG2, NG2), :].rearrange(
                        "d n l -> d (n l)"))
                ktp.append(t)
            for h in range(2):
                kts = [t[:, ts(h, NG * L)] for t in ktp]
                aps = []
                for ti, mt in enumerate(DT):
                    pa = psA.tile([P, NG * L], F32, tag="pa")
                    for si, st in enumerate(DT):
                        nc.tensor.matmul(pa[:mt], lhsT=wcon_sb[si][:st, ds(ti * 128, mt)],
                                         rhs=kts[si][:st], start=(si == 0), stop=(si == 2))
                    aps.append(pa)
                acps = []
                for ti, mt in enumerate(DT):
                    c = ap.tile([P, NG * L], FP8, tag="ac")
                    if ti == 2:
                        nc.vector.tensor_copy(c[:mt], aps[ti][:mt])
                    else:
                        nc.scalar.copy(out=c[:mt], in_=aps[ti][:mt])
                    acps.append(c)
                asqs = []
                for ti, d_ in enumerate(DT):
                    q2 = sq.tile([P, NG * L], BF, tag="asq")
                    nc.vector.tensor_mul(q2[:d_], acps[ti][:d_], acps[ti][:d_])
                    asqs.append(q2)
                pan = psN.tile([P, NG * L], F32, tag="pn")
                for si, st in enumerate(DT):
                    nc.tensor.matmul(pan[:], lhsT=ones_bf[:st, :], rhs=asqs[si][:st],
                                     start=(si == 0), stop=(si == 2))
                raf = rp.tile([P, NG * L], F32, tag="raf")
                nc.vector.reciprocal(raf[:], pan[:])
                # ra = 5/(KS*||A||): folds fp8 scale and the final 10*0.5 factor
                ra = rp.tile([P, NG * L], F32, tag="ra")
                nc.scalar.activation(ra[:], raf[:], AF.Sqrt, scale=25.0 / (KS * KS))
                pc = psC.tile([L, NG * L], F32, tag="pc")
                for n in range(NG):
                    sl = ts(n, L)
                    for si, st in enumerate(DT):
                        nc.tensor.matmul(pc[:, sl], lhsT=kts[si][:st, sl],
                                         rhs=acps[si][:st, sl], start=(si == 0), stop=(si == 2))
                cab = cp.tile([L, NG * L], F32, tag="cab")
                nc.scalar.activation(cab[:], pc[:], AF.Abs)
                m1 = cp.tile([L, NG * L], F32, tag="m1")
                nc.gpsimd.tensor_tensor(out=m1[:], in0=cab[:], in1=ra[:L, :], op=OP.mult)
                nc.gpsimd.tensor_tensor(out=acc[:], in0=acc[:], in1=m1[:], op=OP.add)

        # fold 4 n-slices: accb = 5*sum_n |cos| = 0.5*alphas_con
        f1 = semp.tile([L, L], F32, tag="f1")
        nc.gpsimd.tensor_tensor(out=f1[:], in0=acc[:, ts(0, L)], in1=acc[:, ts(1, L)], op=OP.add)
        f2 = semp.tile([L, L], F32, tag="f2")
        nc.gpsimd.tensor_tensor(out=f2[:], in0=acc[:, ts(2, L)], in1=acc[:, ts(3, L)], op=OP.add)
        accb = semp.tile([L, L], F32, tag="accb")
        nc.gpsimd.tensor_tensor(out=accb[:], in0=f1[:], in1=f2[:], op=OP.add)

        # ------- semantic tail: score, windowed softmax, combine -------
        def st(tag, shape=(L, L), dt_=F32):
            return semp.tile(list(shape), dt_, tag=tag, name=tag)

        xc = st("xc")
        nc.vector.tensor_scalar(out=xc[:], in0=cos_sb[b][:], scalar1=CLIP,
                                scalar2=-CLIP, op0=OP.min, op1=OP.max)
        t_ = st("t")
        nc.scalar.activation(t_[:], xc[:], AF.Abs)
        t2 = st("t2")
        nc.vector.tensor_mul(t2[:], t_[:], t_[:])
        e_ = st("e")
        nc.vector.tensor_scalar(out=e_[:], in0=t2[:], scalar1=A2, scalar2=A0,
                                op0=OP.mult, op1=OP.add)
        o_ = st("o")
        nc.vector.tensor_scalar(out=o_[:], in0=t2[:], scalar1=A3, scalar2=A1,
                                op0=OP.mult, op1=OP.add)
        o2 = st("o2")
        nc.vector.tensor_mul(o2[:], o_[:], t_[:])
        pl = st("pl")
        nc.gpsimd.tensor_tensor(out=pl[:], in0=e_[:], in1=o2[:], op=OP.add)
        sm = st("sm")
        nc.scalar.activation(sm[:], t_[:], AF.Sqrt, bias=1.0, scale=-1.0)
        w_ = st("w")
        nc.vector.tensor_mul(w_[:], sm[:], pl[:])
        z_ = st("z")
        nc.vector.tensor_scalar(out=z_[:], in0=w_[:], scalar1=-1.0, scalar2=0.5,
                                op0=OP.mult, op1=OP.add)
        sg = st("sg")
        nc.scalar.sign(sg[:], xc[:])
        m_ = st("m")
        nc.vector.tensor_mul(m_[:], sg[:], z_[:])
        # sM = (score-0.5)*fm + fmb  (score = m_+0.5; masked rows get -NEG)
        s1 = st("s1")
        nc.gpsimd.tensor_tensor(out=s1[:], in0=m_[:], in1=fm_sb[b][:], op=OP.mult)
        sM = st("sM")
        nc.gpsimd.tensor_tensor(out=sM[:], in0=s1[:], in1=fmb_sb[b][:], op=OP.add)
        ex = st("ex")
        rsum = st("rsum", (L, 1))
        nc.scalar.activation(ex[:], sM[:], AF.Exp, accum_out=rsum[:])
        rs2 = st("rs2", (L, 1))
        nc.vector.tensor_scalar(out=rs2[:], in0=rsum[:], scalar1=1e-8, scalar2=None,
                                op0=OP.max)
        rr = st("rr", (L, 1))
        nc.vector.reciprocal(rr[:], rs2[:])
        rrh = st("rrh", (L, 1))
        nc.vector.tensor_scalar(out=rrh[:], in0=rr[:], scalar1=0.5, scalar2=None,
                                op0=OP.mult)
        c2 = st("c2")
        nc.vector.tensor_scalar(out=c2[:], in0=ex[:], scalar1=rrh[:], scalar2=None,
                                op0=OP.mult)
        c3 = st("c3")
        nc.gpsimd.tensor_tensor(out=c3[:], in0=accb[:], in1=c2[:], op=OP.add)
        nc.vector.tensor_mul(oball[:, ts(b, L)], c3[:], fm_sb[b][:])

    # band-compact output: rows 8g..8g+7 share a 28-wide window covering all
    # their |j-k|<=10 bands, so one DMA per 8-row group suffices.
    obv = oball[:].rearrange("p (b k) -> p b k", b=BPC)
    for g in range((L + GB - 1) // GB):
        j0 = g * GB
        rows = min(GB, L - j0)
        s = min(max(j0 - WP, 0), L - BW)
        nc.sync.dma_start(out=out[ds(j0, rows)], in_=obv[ds(j0, rows), :, ds(s, BW)])


_NC_CACHE = None
_RUNNER_CACHE = None


def _get_nc():
    global _NC_CACHE
    if _NC_CACHE is None:
        _NC_CACHE = _build_nc()
    return _NC_CACHE


def _get_runner():
    """Persistent jit runner: traced once, dummy output buffers device-resident."""
    global _RUNNER_CACHE
    if _RUNNER_CACHE is not None:
        return _RUNNER_CACHE

    import jax
    from jax.sharding import Mesh, PartitionSpec, NamedSharding
    from jax.experimental.shard_map import shard_map
    from concourse.bass2jax import (_bass_exec_p, install_neuronx_cc_hook,
                                    partition_id_tensor)

    nc = _get_nc()
    install_neuronx_cc_hook()

    in_names, out_names, out_avals = [], [], []
    partition_name = nc.partition_id_tensor.name if nc.partition_id_tensor else None
    for alloc in nc.m.functions[0].allocations:
        if not isinstance(alloc, mybir.MemoryLocationSet):
            continue
        name = alloc.memorylocations[0].name
        if alloc.kind == "ExternalInput":
            if name != partition_name:
                in_names.append(name)
        elif alloc.kind == "ExternalOutput":
            out_names.append(name)
            out_avals.append(jax.core.ShapedArray(
                tuple(alloc.tensor_shape), mybir.dt.np(alloc.dtype)))
    n_params = len(in_names)
    all_in_names = list(in_names) + list(out_names)
    if partition_name is not None:
        all_in_names.append(partition_name)

    def _body(*args):
        operands = list(args)
        if partition_name is not None:
            operands.append(partition_id_tensor())
        return tuple(_bass_exec_p.bind(
            *operands,
            out_avals=tuple(out_avals),
            in_names=tuple(all_in_names),
            out_names=tuple(out_names),
            lowering_input_output_aliases=(),
            sim_require_finite=True,
            sim_require_nnan=True,
            nc=nc,
        ))

    devices = jax.devices()[:NCORES]
    mesh = Mesh(np.asarray(devices), ("core",))
    nspec = (PartitionSpec("core"),) * (n_params + len(out_names))
    donate = tuple(range(n_params, n_params + len(out_names)))
    sharded = jax.jit(shard_map(
        _body, mesh=mesh, in_specs=nspec,
        out_specs=(PartitionSpec("core"),) * len(out_names), check_rep=False),
        donate_argnums=donate, keep_unused=True)
    # initial donated output buffers, device-resident; steady state donates
    # the previous call's outputs instead, so no zero-buffers ship per call.
    shard = NamedSharding(mesh, PartitionSpec("core"))
    dummies = [jax.device_put(
        np.zeros((NCORES * a.shape[0], *a.shape[1:]), a.dtype), shard)
        for a in out_avals]
    _RUNNER_CACHE = [sharded, in_names, out_names, out_avals, dummies]
    return _RUNNER_CACHE


def _make_in_maps(node_features, knowledge, weight_sem, weight_con, text_len):
    """Returns the global (concatenated-over-cores) input arrays, in order."""
    e4 = ml_dtypes.float8_e4m3
    nf = np.asarray(node_features, np.float32)
    K = np.asarray(knowledge, np.float32)
    # host normalization: cosine similarity is invariant to per-vector scale
    nfh = (nf / np.linalg.norm(nf, axis=-1, keepdims=True) * KS).astype(e4)
    Kh = (K / np.linalg.norm(K, axis=-1, keepdims=True) * KS).astype(e4)
    wsemT_ = np.ascontiguousarray(
        np.asarray(weight_sem, np.float32).T * WS).astype(e4)
    wcon_ = np.ascontiguousarray(
        np.asarray(weight_con, np.float32) * WS).astype(e4)
    tl = np.asarray(text_len).astype(np.int64)
    j = np.arange(L)[:, None]
    k = np.arange(L)[None, :]
    win = (k >= j - WP) & (k <= j + WF)
    cur = tl[:, None, None]
    fm = (win[None] & (k[None] <= cur - 1) & (j[None] < cur)).astype(e4)

    kT_g = np.ascontiguousarray(
        Kh.transpose(0, 3, 2, 1))                      # [B, D, N, L] fp8
    nfT_g = np.ascontiguousarray(
        nfh.transpose(2, 0, 1))                        # [G, B, L] fp8
    # global layouts: per-core shard is axis-0 slice of each array
    arrs = dict(
        kT=kT_g,                                       # [8*BPC, D, N, L]
        nfT=np.ascontiguousarray(
            nfT_g.reshape(G, NCORES, BPC, L).transpose(1, 0, 2, 3)
        ).reshape(NCORES * G, BPC, L),                 # [8*G, BPC, L]
        wsemT=np.ascontiguousarray(
            np.broadcast_to(wsemT_, (NCORES, G, G))).reshape(NCORES * G, G),
        wcon=np.ascontiguousarray(
            np.broadcast_to(wcon_, (NCORES, D, D))).reshape(NCORES * D, D),
        fmask=np.ascontiguousarray(fm),                # [8*BPC, L, L]
    )
    return arrs


def run_on_hw(in_maps, trace=False, **kw):
    global _RUNNER_CACHE
    last_exc = None
    for attempt in range(3):
        try:
            state = _get_runner()
            sharded, in_names, out_names, out_avals, dummies = state
            args = [in_maps[n] for n in in_names]
            outs = sharded(*args, *dummies)
            outs_np = [np.asarray(o) for o in outs]
            state[4] = list(outs)  # donate these (device-resident) next call
            results = []
            for c in range(NCORES):
                results.append({
                    n: outs_np[i].reshape(NCORES, *out_avals[i].shape)[c]
                    for i, n in enumerate(out_names)})
            return SimpleNamespace(results=results, exec_time_ns=None)
        except Exception as e:  # rare transient NRT exec failure: reset + retry
            last_exc = e
            _RUNNER_CACHE = None
            import time as _time
            _time.sleep(2.0 * (attempt + 1))
    raise last_exc


_BAND_IDX = None


def _unpack_band(band_lbc):
    """[L, BPC, BW] band -> [BPC, L, L] full (zeros outside the band)."""
    global _BAND_IDX
    if _BAND_IDX is None:
        s = np.minimum(np.maximum((np.arange(L) // GB) * GB - WP, 0), L - BW)
        _BAND_IDX = (s[:, None] + np.arange(BW))[None]  # [1, L, BW]
    full = np.zeros((BPC, L, L), np.float32)
    np.put_along_axis(full, _BAND_IDX, band_lbc.transpose(1, 0, 2).astype(np.float32),
                      axis=2)
    return full


def kernel(node_features, knowledge, anew, weight_sem, weight_con, text_len):
    del anew  # strictly-positive affinity scale cancels in cosine similarity
    in_maps = _make_in_maps(node_features, knowledge, weight_sem, weight_con, text_len)
    res = run_on_hw(in_maps).results
    return np.concatenate([_unpack_band(np.asarray(r["out"])) for r in res], axis=0)
